# revision 11
# baseline (speedup 1.0000x reference)
"""Trainium2 Bass kernel for nn_EquivariantScalarMLP (data-parallel over 8 cores).

Per-core layout (8192 samples, 64 tiles x 128 samples), feature-major chain:
  - Host pre-transposes the scalar block and pre-rearranges the equivariant
    block to (d-major, i-contiguous) bf16, then PACKS both into one
    per-tile-contiguous DRAM block (xin: [bpc, 512] where rows n0:n0+128 are
    tile t's [128 partition-lines x 1024 B]). The conditioning (+ones bias
    row) is packed per tile as cTp: [nt, 65, 128]. This keeps every
    per-tile DMA contiguous with >=512 B per partition line - the original
    column-gather layout (256 B / 130 B descriptors) made the kernel
    DMA-bound on hardware (~15 us/tile of DMA vs ~2 us modeled).
  - FiLM1/FiLM2 gammas/betas computed feature-major with film weights as the
    PE stationary; scalar chain s0T -> s1T -> s2T -> s3T stays feature-major,
    so fc1/inv/wemb all use s*T chunks directly as stationary operands.
  - Hypernet weights w = s3 @ wemb land batch-on-partition in PSUM; wemb
    columns are host-permuted to (o-major, i) per irrep block so the DVE mix
    runs at 2x: products [p,(o,d),i] via one broadcast mul per irrep, then an
    in-place log2 halving tree over i (bf16, innermost step-1 throughout).
  - e3nn 1/sqrt(32) norm folded into wemb on host; film biases folded via an
    extra ones row on condT (k=65 matmul).
"""

import numpy as np
import ml_dtypes

B = 65536
NCORES = 8
BPC = B // NCORES          # 8192 samples per core
NT = 64
S_IN = 256
S_OUT = 128
MUL = 32
COND = 64
D1, D2 = 3, 5

_cache = {}


def _build(nt=64, reps=1):
    from contextlib import ExitStack
    import concourse.bass as bass
    from concourse import bacc
    import concourse.tile as tile
    import concourse.mybir as mybir

    bpc = nt * 128
    f32 = mybir.dt.float32
    bf16 = mybir.dt.bfloat16
    AF = mybir.ActivationFunctionType

    nc = bacc.Bacc()

    xin_d = nc.dram_tensor("xin", [bpc, 512], bf16, kind="ExternalInput")
    cTp_d = nc.dram_tensor("cTp", [nt, COND + 1, 128], bf16, kind="ExternalInput")
    f12w_d = nc.dram_tensor("f12w", [COND + 1, 512], bf16, kind="ExternalInput")
    b1fc_d = nc.dram_tensor("b1fc", [COND + 1, 256], bf16, kind="ExternalInput")
    b2iv_d = nc.dram_tensor("b2iv", [COND + 1, 128], bf16, kind="ExternalInput")
    b2wb_d = nc.dram_tensor("b2wb", [COND + 1, 2048], bf16, kind="ExternalInput")
    fcw_d = nc.dram_tensor("fcw", [S_IN, S_IN], bf16, kind="ExternalInput")
    invw_d = nc.dram_tensor("invw", [S_IN, S_OUT], bf16, kind="ExternalInput")
    wembw_d = nc.dram_tensor("wembw", [S_IN, 2048], bf16, kind="ExternalInput")
    out_d = nc.dram_tensor("out", [bpc, 384], bf16, kind="ExternalOutput")

    with tile.TileContext(nc) as tc, ExitStack() as ctx:
        wpool = ctx.enter_context(tc.tile_pool(name="weights", bufs=1))
        # --- persistent weights in SBUF ---
        f12w = wpool.tile([COND + 1, 512], bf16)
        nc.sync.dma_start(f12w[:], f12w_d[:])
        b1fc = wpool.tile([COND + 1, 256], bf16)
        nc.sync.dma_start(b1fc[:], b1fc_d[:])
        b2iv = wpool.tile([COND + 1, 128], bf16)
        nc.sync.dma_start(b2iv[:], b2iv_d[:])
        b2wb = wpool.tile([COND + 1, 2048], bf16)
        nc.sync.dma_start(b2wb[:], b2wb_d[:])
        # k-major stores: chunk c holds rows [128c, 128c+128) of the weight
        fcw = wpool.tile([128, 512], bf16)
        nc.sync.dma_start(fcw[:].rearrange("k (c m) -> k c m", c=2),
                          fcw_d.rearrange("(c k) m -> k c m", c=2))
        invw = wpool.tile([128, 256], bf16)
        nc.sync.dma_start(invw[:].rearrange("k (c m) -> k c m", c=2),
                          invw_d.rearrange("(c k) m -> k c m", c=2))
        wemb = wpool.tile([128, 4096], bf16)
        nc.sync.dma_start(wemb[:].rearrange("k (c m) -> k c m", c=2),
                          wembw_d.rearrange("(c k) m -> k c m", c=2))

        inp = ctx.enter_context(tc.tile_pool(name="inp", bufs=3))
        act = ctx.enter_context(tc.tile_pool(name="act", bufs=2))
        wsbp = ctx.enter_context(tc.tile_pool(name="wsbp", bufs=2))
        prodp = ctx.enter_context(tc.tile_pool(name="prodp", bufs=2))
        outp = ctx.enter_context(tc.tile_pool(name="outp", bufs=3))
        # PSUM pools (bank-granular: 8 x 2KB per partition total)
        ps_gb = ctx.enter_context(tc.tile_pool(name="ps_gb", bufs=1, space="PSUM"))
        ps_s2 = ctx.enter_context(tc.tile_pool(name="ps_s2", bufs=1, space="PSUM"))
        ps_iv = ctx.enter_context(tc.tile_pool(name="ps_iv", bufs=1, space="PSUM"))
        ps_w = ctx.enter_context(tc.tile_pool(name="ps_w", bufs=1, space="PSUM"))

        def stage_a(t):
            n0 = t * 128
            # one contiguous 128 KiB load: cols 0:256 = s0T (feature-major
            # scalars, [p, c, b] with f = 128c+p), cols 256:512 = xr
            xin = inp.tile([128, 512], bf16, tag="xin")
            nc.sync.dma_start(xin[:], xin_d[n0:n0 + 128, :])
            s0T = xin[:, 0:256].rearrange("p (c b) -> p c b", c=2)
            xr = xin[:, 256:512]
            cT = inp.tile([COND + 1, 128], bf16, tag="cT")
            nc.sync.dma_start(cT[:], cTp_d[t])

            # --- films feature-major, gammas only (betas folded on host) ---
            gb = ps_gb.tile([128, 512], f32, tag="gb")
            for m in range(4):
                nc.tensor.matmul(gb[:, 128 * m:128 * (m + 1)],
                                 f12w[:, 128 * m:128 * (m + 1)], cT[:],
                                 start=True, stop=True)
            gam1 = act.tile([128, 256], bf16, tag="gam1")
            gam2 = act.tile([128, 256], bf16, tag="gam2")
            nc.scalar.activation(gam1[:], gb[:, 0:256], AF.Copy)
            nc.scalar.activation(gam2[:], gb[:, 256:512], AF.Copy)

            # --- FiLM1 gamma on DVE; beta1 enters via b1fc matmul below ---
            s1T = act.tile([128, 256], bf16, tag="s1T")
            nc.gpsimd.tensor_mul(s1T[:], s0T.rearrange("p c b -> p (c b)"), gam1[:])

            # --- fc1 + folded beta1: s2T[m=feat, b] ---
            s2ps = ps_s2.tile([128, 256], f32, tag="s2ps")
            for m in range(2):
                nc.tensor.matmul(s2ps[:, 128 * m:128 * (m + 1)],
                                 b1fc[:, 128 * m:128 * (m + 1)], cT[:],
                                 start=True, stop=False)
                for c in range(2):
                    nc.tensor.matmul(s2ps[:, 128 * m:128 * (m + 1)],
                                     fcw[:, 256 * c + 128 * m:256 * c + 128 * (m + 1)],
                                     s1T[:, 128 * c:128 * (c + 1)],
                                     start=False, stop=(c == 1))
            s2T = act.tile([128, 256], bf16, tag="s2T")
            nc.scalar.activation(s2T[:], s2ps[:], AF.Copy)

            # --- FiLM2 gamma on DVE; beta2 folded into inv/wemb below ---
            s3T = act.tile([128, 256], bf16, tag="s3T")
            nc.gpsimd.tensor_mul(s3T[:], s2T[:], gam2[:])

            ostage = outp.tile([128, 384], bf16, tag="ostage")

            # --- inv (batch-major) + hypernet w, sharing s3T stationary ---
            invps = ps_iv.tile([128, 128], f32, tag="invps")
            wps0 = ps_w.tile([128, 1024], f32, tag="wps0")
            wps1 = ps_w.tile([128, 1024], f32, tag="wps1")
            wps = [wps0, wps1]
            nc.tensor.matmul(invps[:], cT[:], b2iv[:], start=True, stop=False)
            for h in range(2):
                for q in range(2):
                    nc.tensor.matmul(wps[h][:, 512 * q:512 * (q + 1)], cT[:],
                                     b2wb[:, 1024 * h + 512 * q:1024 * h + 512 * (q + 1)],
                                     start=True, stop=False)
            for c in range(2):
                nc.tensor.matmul(invps[:], s3T[:, 128 * c:128 * (c + 1)],
                                 invw[:, 128 * c:128 * (c + 1)],
                                 start=False, stop=(c == 1))
                for h in range(2):
                    for q in range(2):
                        nc.tensor.matmul(
                            wps[h][:, 512 * q:512 * (q + 1)],
                            s3T[:, 128 * c:128 * (c + 1)],
                            wemb[:, 2048 * c + 1024 * h + 512 * q:
                                 2048 * c + 1024 * h + 512 * (q + 1)],
                            start=False, stop=(c == 1))
            nc.scalar.activation(ostage[:, 0:128], invps[:], AF.Copy)
            wsb = wsbp.tile([128, 2048], bf16, tag="wsb")
            nc.scalar.activation(wsb[:, 0:1024], wps0[:], AF.Copy)
            nc.scalar.activation(wsb[:, 1024:2048], wps1[:], AF.Copy)
            return wsb, xr, ostage

        def stage_b(t, wsb, xr, ostage):
            n0 = t * 128
            # --- equivariant mix on DVE: products then halving tree over i ---
            # prod rows: 96 = (o,d) for l=1, then 160 = (o,d) for l=2
            # engine split balances DVE (~0.54 ns/elem) vs Pool (~0.92):
            # DVE: products l2 + step1(all) + step2-l2; Pool: products l1 +
            # step2-l1 + step3 + step4 + final (plus s1T/s3T in stage_a)
            prod = prodp.tile([128, 256, 32], bf16, tag="prod")
            w1v = wsb[:, 0:1024].rearrange("p (o i) -> p o i", o=32).unsqueeze(2).broadcast_to((128, 32, 3, 32))
            x1v = xr[:, 0:96].rearrange("p (d i) -> p d i", d=3).unsqueeze(1).broadcast_to((128, 32, 3, 32))
            nc.gpsimd.tensor_mul(prod[:, 0:96, :].rearrange("p (o d) i -> p o d i", o=32), w1v, x1v)
            w2v = wsb[:, 1024:2048].rearrange("p (o i) -> p o i", o=32).unsqueeze(2).broadcast_to((128, 32, 5, 32))
            x2v = xr[:, 96:256].rearrange("p (d i) -> p d i", d=5).unsqueeze(1).broadcast_to((128, 32, 5, 32))
            nc.vector.tensor_mul(prod[:, 96:256, :].rearrange("p (o d) i -> p o d i", o=32), w2v, x2v)
            nc.vector.tensor_add(prod[:, :, 0:16], prod[:, :, 0:16],
                                 prod[:, :, 16:32])
            nc.vector.tensor_add(prod[:, 96:256, 0:8], prod[:, 96:256, 0:8],
                                 prod[:, 96:256, 8:16])
            nc.gpsimd.tensor_add(prod[:, 0:96, 0:8], prod[:, 0:96, 0:8],
                                 prod[:, 0:96, 8:16])
            nc.gpsimd.tensor_add(prod[:, :, 0:4], prod[:, :, 0:4], prod[:, :, 4:8])
            nc.gpsimd.tensor_add(prod[:, :, 0:2], prod[:, :, 0:2], prod[:, :, 2:4])
            nc.gpsimd.tensor_add(ostage[:, 128:384], prod[:, :, 0], prod[:, :, 1])

            nc.scalar.dma_start(out_d[n0:n0 + 128, :], ostage[:])

        # reps>1 is used only by the timing harness: the same (static) body is
        # unrolled `reps` times in one NEFF so per-dispatch overhead can be
        # subtracted out via a two-point slope.
        for _ in range(reps):
            pending = None
            for t in range(nt):
                cur = stage_a(t)
                if pending is not None:
                    stage_b(t - 1, *pending)
                pending = cur
            stage_b(nt - 1, *pending)

    return nc


def _prep_shared(film1_w, film1_b, fc1_w, film2_w, film2_b, inv_w, wemb_w):
    bf = ml_dtypes.bfloat16
    norm = np.float32(1.0 / np.sqrt(MUL))
    f1w = np.asarray(film1_w, np.float32); f1b = np.asarray(film1_b, np.float32)
    f2w = np.asarray(film2_w, np.float32); f2b = np.asarray(film2_b, np.float32)
    fcw = np.asarray(fc1_w, np.float32)
    invw = np.asarray(inv_w, np.float32)
    # wemb: fold norm; permute each irrep block from (i-major o) to (o-major i)
    wm = np.asarray(wemb_w, np.float32) * norm
    wp = np.empty_like(wm)
    wp[:, 0:1024] = wm[:, 0:1024].reshape(-1, MUL, MUL).transpose(0, 2, 1).reshape(-1, 1024)
    wp[:, 1024:2048] = wm[:, 1024:2048].reshape(-1, MUL, MUL).transpose(0, 2, 1).reshape(-1, 1024)
    # gammas with bias row: [65, 512] = (F1g; b1g) || (F2g; b2g)
    f12w = np.zeros((COND + 1, 512), np.float32)
    f12w[:COND, 0:256] = f1w[:, 0:S_IN]
    f12w[COND, 0:256] = f1b[0:S_IN]
    f12w[:COND, 256:512] = f2w[:, 0:S_IN]
    f12w[COND, 256:512] = f2b[0:S_IN]
    # beta folds (with bias rows), f32 host matmuls then bf16
    b1 = np.zeros((COND + 1, S_IN), np.float32)
    b1[:COND] = f1w[:, S_IN:]; b1[COND] = f1b[S_IN:]
    b2 = np.zeros((COND + 1, S_IN), np.float32)
    b2[:COND] = f2w[:, S_IN:]; b2[COND] = f2b[S_IN:]
    return {
        "f12w": f12w.astype(bf),
        "b1fc": (b1 @ fcw).astype(bf),
        "b2iv": (b2 @ invw).astype(bf),
        "b2wb": (b2 @ wp).astype(bf),
        "fcw": fcw.astype(bf),
        "invw": invw.astype(bf),
        "wembw": wp.astype(bf),
    }


def _prep_per_core(features, conditioning_tensor, nt=NT):
    """Pack activations into per-tile-contiguous blocks (see module docstring).

    Returns (xin, cTp): xin [NCORES, bpc, 512] bf16, cTp [NCORES, nt, 65, 128].
    """
    bf = ml_dtypes.bfloat16
    feats = np.asarray(features, np.float32)
    conds = np.asarray(conditioning_tensor, np.float32)
    ncr, bpc = NCORES, nt * 128

    # s0T block: [core, tile, p, (c b)] with value scalars[sample=n0+b, f=128c+p]
    sc = feats[:, :S_IN].astype(bf).reshape(ncr, nt, 128, 2, 128)  # [.., b, c, p]
    s0 = np.ascontiguousarray(sc.transpose(0, 1, 4, 3, 2)).reshape(ncr, nt, 128, 256)

    # xr block: equivariant features, (d-major, i) per irrep, per sample
    xe = np.empty((B, 256), bf)
    xe[:, 0:96] = feats[:, 256:352].reshape(-1, MUL, D1).transpose(0, 2, 1).reshape(-1, 96).astype(bf)
    xe[:, 96:256] = feats[:, 352:512].reshape(-1, MUL, D2).transpose(0, 2, 1).reshape(-1, 160).astype(bf)
    xr = xe.reshape(ncr, nt, 128, 256)

    xin = np.concatenate([s0, xr], axis=-1).reshape(ncr, bpc, 512)

    # cTp: [core, tile, 65, 128] with ones bias row
    cT = np.empty((COND + 1, B), bf)
    cT[:COND] = conds.T.astype(bf)
    cT[COND] = np.ones((B,), bf)
    cTp = np.ascontiguousarray(
        cT.reshape(COND + 1, ncr, nt, 128).transpose(1, 2, 0, 3))
    return np.ascontiguousarray(xin), cTp


def kernel(features, conditioning_tensor, film1_w, film1_b, fc1_w,
           film2_w, film2_b, inv_w, wemb_w):
    from concourse.bass_utils import run_bass_kernel_spmd

    if "nc" not in _cache:
        nc = _build()
        if not nc.is_finalized():
            nc.finalize()
        _cache["nc"] = nc
    nc = _cache["nc"]

    shared = _prep_shared(film1_w, film1_b, fc1_w, film2_w, film2_b, inv_w, wemb_w)
    xin, cTp = _prep_per_core(features, conditioning_tensor)

    in_maps = []
    for i in range(NCORES):
        m = dict(shared)
        m["xin"] = xin[i]
        m["cTp"] = cTp[i]
        in_maps.append(m)

    import os
    trace = bool(int(os.environ.get("KERNEL_TRACE", "0")))
    res = run_bass_kernel_spmd(nc, in_maps, core_ids=list(range(NCORES)), trace=trace)
    _cache["last"] = res
    return np.concatenate([r["out"] for r in res.results], axis=0).astype(np.float32)


# revision 12
# speedup vs baseline: 1.3300x; 1.3300x over previous
"""Trainium2 Bass kernel for nn_EquivariantScalarMLP (data-parallel over 8 cores).

Per-core layout (8192 samples, 64 tiles x 128 samples), feature-major chain:
  - Host pre-transposes the scalar block and pre-rearranges the equivariant
    block to (d-major, i-contiguous) bf16, then PACKS both into one
    per-tile-contiguous DRAM block (xin: [bpc, 512] where rows n0:n0+128 are
    tile t's [128 partition-lines x 1024 B]). The conditioning (+ones bias
    row) is packed per tile as cTp: [nt, 65, 128]. This keeps every
    per-tile DMA contiguous with >=512 B per partition line - the original
    column-gather layout (256 B / 130 B descriptors) made the kernel
    DMA-bound on hardware (~15 us/tile of DMA vs ~2 us modeled).
  - FiLM1/FiLM2 gammas/betas computed feature-major with film weights as the
    PE stationary; scalar chain s0T -> s1T -> s2T -> s3T stays feature-major,
    so fc1/inv/wemb all use s*T chunks directly as stationary operands.
  - Hypernet weights w = s3 @ wemb land batch-on-partition in PSUM; wemb
    columns are host-permuted to (o-major, i) per irrep block so the DVE mix
    runs at 2x: products [p,(o,d),i] via one broadcast mul per irrep, then an
    in-place log2 halving tree over i (bf16, innermost step-1 throughout).
  - e3nn 1/sqrt(32) norm folded into wemb on host; film biases folded via an
    extra ones row on condT (k=65 matmul).
"""

import numpy as np
import ml_dtypes

B = 65536
NCORES = 8
BPC = B // NCORES          # 8192 samples per core
NT = 64
S_IN = 256
S_OUT = 128
MUL = 32
COND = 64
D1, D2 = 3, 5

_cache = {}


def _build(nt=64, reps=1):
    from contextlib import ExitStack
    import concourse.bass as bass
    from concourse import bacc
    import concourse.tile as tile
    import concourse.mybir as mybir

    bpc = nt * 128
    f32 = mybir.dt.float32
    bf16 = mybir.dt.bfloat16
    AF = mybir.ActivationFunctionType

    nc = bacc.Bacc()

    xin_d = nc.dram_tensor("xin", [bpc, 512], bf16, kind="ExternalInput")
    cTp_d = nc.dram_tensor("cTp", [nt, COND + 1, 128], bf16, kind="ExternalInput")
    f12w_d = nc.dram_tensor("f12w", [COND + 1, 512], bf16, kind="ExternalInput")
    b1fc_d = nc.dram_tensor("b1fc", [COND + 1, 256], bf16, kind="ExternalInput")
    b2iv_d = nc.dram_tensor("b2iv", [COND + 1, 128], bf16, kind="ExternalInput")
    b2wb_d = nc.dram_tensor("b2wb", [COND + 1, 2048], bf16, kind="ExternalInput")
    fcw_d = nc.dram_tensor("fcw", [S_IN, S_IN], bf16, kind="ExternalInput")
    invw_d = nc.dram_tensor("invw", [S_IN, S_OUT], bf16, kind="ExternalInput")
    wembw_d = nc.dram_tensor("wembw", [S_IN, 2048], bf16, kind="ExternalInput")
    out_d = nc.dram_tensor("out", [bpc, 384], bf16, kind="ExternalOutput")

    with tile.TileContext(nc) as tc, ExitStack() as ctx:
        wpool = ctx.enter_context(tc.tile_pool(name="weights", bufs=1))
        # --- persistent weights in SBUF ---
        f12w = wpool.tile([COND + 1, 512], bf16)
        nc.sync.dma_start(f12w[:], f12w_d[:])
        b1fc = wpool.tile([COND + 1, 256], bf16)
        nc.sync.dma_start(b1fc[:], b1fc_d[:])
        b2iv = wpool.tile([COND + 1, 128], bf16)
        nc.sync.dma_start(b2iv[:], b2iv_d[:])
        b2wb = wpool.tile([COND + 1, 2048], bf16)
        nc.sync.dma_start(b2wb[:], b2wb_d[:])
        # k-major stores: chunk c holds rows [128c, 128c+128) of the weight
        fcw = wpool.tile([128, 512], bf16)
        nc.sync.dma_start(fcw[:].rearrange("k (c m) -> k c m", c=2),
                          fcw_d.rearrange("(c k) m -> k c m", c=2))
        invw = wpool.tile([128, 256], bf16)
        nc.sync.dma_start(invw[:].rearrange("k (c m) -> k c m", c=2),
                          invw_d.rearrange("(c k) m -> k c m", c=2))
        wemb = wpool.tile([128, 4096], bf16)
        nc.sync.dma_start(wemb[:].rearrange("k (c m) -> k c m", c=2),
                          wembw_d.rearrange("(c k) m -> k c m", c=2))

        inp = ctx.enter_context(tc.tile_pool(name="inp", bufs=3))
        act = ctx.enter_context(tc.tile_pool(name="act", bufs=2))
        wsbp = ctx.enter_context(tc.tile_pool(name="wsbp", bufs=2))
        prodp = ctx.enter_context(tc.tile_pool(name="prodp", bufs=2))
        outp = ctx.enter_context(tc.tile_pool(name="outp", bufs=3))
        # PSUM pools (bank-granular: 8 x 2KB per partition total)
        ps_gb = ctx.enter_context(tc.tile_pool(name="ps_gb", bufs=1, space="PSUM"))
        ps_s2 = ctx.enter_context(tc.tile_pool(name="ps_s2", bufs=1, space="PSUM"))
        ps_iv = ctx.enter_context(tc.tile_pool(name="ps_iv", bufs=1, space="PSUM"))
        ps_w = ctx.enter_context(tc.tile_pool(name="ps_w", bufs=1, space="PSUM"))

        def stage_a(t):
            n0 = t * 128
            # one contiguous 128 KiB load: cols 0:256 = s0T (feature-major
            # scalars, [p, c, b] with f = 128c+p), cols 256:512 = xr
            xin = inp.tile([128, 512], bf16, tag="xin")
            eng_in = nc.sync if t % 2 == 0 else nc.scalar
            eng_in.dma_start(xin[:], xin_d[n0:n0 + 128, :])
            s0T = xin[:, 0:256].rearrange("p (c b) -> p c b", c=2)
            xr = xin[:, 256:512]
            cT = inp.tile([COND + 1, 128], bf16, tag="cT")
            nc.sync.dma_start(cT[:], cTp_d[t])

            # --- films feature-major, gammas only (betas folded on host) ---
            gb = ps_gb.tile([128, 512], f32, tag="gb")
            for m in range(4):
                nc.tensor.matmul(gb[:, 128 * m:128 * (m + 1)],
                                 f12w[:, 128 * m:128 * (m + 1)], cT[:],
                                 start=True, stop=True)
            gam1 = act.tile([128, 256], bf16, tag="gam1")
            gam2 = act.tile([128, 256], bf16, tag="gam2")
            nc.scalar.activation(gam1[:], gb[:, 0:256], AF.Copy)
            nc.scalar.activation(gam2[:], gb[:, 256:512], AF.Copy)

            # --- FiLM1 gamma on DVE; beta1 enters via b1fc matmul below ---
            s1T = act.tile([128, 256], bf16, tag="s1T")
            nc.gpsimd.tensor_mul(s1T[:], s0T.rearrange("p c b -> p (c b)"), gam1[:])

            # --- fc1 + folded beta1: s2T[m=feat, b] ---
            s2ps = ps_s2.tile([128, 256], f32, tag="s2ps")
            for m in range(2):
                nc.tensor.matmul(s2ps[:, 128 * m:128 * (m + 1)],
                                 b1fc[:, 128 * m:128 * (m + 1)], cT[:],
                                 start=True, stop=False)
                for c in range(2):
                    nc.tensor.matmul(s2ps[:, 128 * m:128 * (m + 1)],
                                     fcw[:, 256 * c + 128 * m:256 * c + 128 * (m + 1)],
                                     s1T[:, 128 * c:128 * (c + 1)],
                                     start=False, stop=(c == 1))
            s2T = act.tile([128, 256], bf16, tag="s2T")
            nc.scalar.activation(s2T[:], s2ps[:], AF.Copy)

            # --- FiLM2 gamma on DVE; beta2 folded into inv/wemb below ---
            s3T = act.tile([128, 256], bf16, tag="s3T")
            nc.gpsimd.tensor_mul(s3T[:], s2T[:], gam2[:])

            ostage = outp.tile([128, 384], bf16, tag="ostage")

            # --- inv (batch-major) + hypernet w, sharing s3T stationary ---
            invps = ps_iv.tile([128, 128], f32, tag="invps")
            wps0 = ps_w.tile([128, 1024], f32, tag="wps0")
            wps1 = ps_w.tile([128, 1024], f32, tag="wps1")
            wps = [wps0, wps1]
            nc.tensor.matmul(invps[:], cT[:], b2iv[:], start=True, stop=False)
            for h in range(2):
                for q in range(2):
                    nc.tensor.matmul(wps[h][:, 512 * q:512 * (q + 1)], cT[:],
                                     b2wb[:, 1024 * h + 512 * q:1024 * h + 512 * (q + 1)],
                                     start=True, stop=False)
            for c in range(2):
                nc.tensor.matmul(invps[:], s3T[:, 128 * c:128 * (c + 1)],
                                 invw[:, 128 * c:128 * (c + 1)],
                                 start=False, stop=(c == 1))
                for h in range(2):
                    for q in range(2):
                        nc.tensor.matmul(
                            wps[h][:, 512 * q:512 * (q + 1)],
                            s3T[:, 128 * c:128 * (c + 1)],
                            wemb[:, 2048 * c + 1024 * h + 512 * q:
                                 2048 * c + 1024 * h + 512 * (q + 1)],
                            start=False, stop=(c == 1))
            nc.scalar.activation(ostage[:, 0:128], invps[:], AF.Copy)
            wsb = wsbp.tile([128, 2048], bf16, tag="wsb")
            nc.scalar.activation(wsb[:, 0:1024], wps0[:], AF.Copy)
            nc.scalar.activation(wsb[:, 1024:2048], wps1[:], AF.Copy)
            return wsb, xr, ostage

        def stage_b(t, wsb, xr, ostage):
            n0 = t * 128
            # --- equivariant mix on DVE: products then halving tree over i ---
            # prod rows: 96 = (o,d) for l=1, then 160 = (o,d) for l=2
            # engine split balances DVE (~0.54 ns/elem) vs Pool (~0.92):
            # DVE: products l2 + step1(all) + step2-l2; Pool: products l1 +
            # step2-l1 + step3 + step4 + final (plus s1T/s3T in stage_a)
            prod = prodp.tile([128, 256, 32], bf16, tag="prod")
            w1v = wsb[:, 0:1024].rearrange("p (o i) -> p o i", o=32).unsqueeze(2).broadcast_to((128, 32, 3, 32))
            x1v = xr[:, 0:96].rearrange("p (d i) -> p d i", d=3).unsqueeze(1).broadcast_to((128, 32, 3, 32))
            nc.vector.tensor_mul(prod[:, 0:96, :].rearrange("p (o d) i -> p o d i", o=32), w1v, x1v)
            w2v = wsb[:, 1024:2048].rearrange("p (o i) -> p o i", o=32).unsqueeze(2).broadcast_to((128, 32, 5, 32))
            x2v = xr[:, 96:256].rearrange("p (d i) -> p d i", d=5).unsqueeze(1).broadcast_to((128, 32, 5, 32))
            nc.vector.tensor_mul(prod[:, 96:256, :].rearrange("p (o d) i -> p o d i", o=32), w2v, x2v)
            nc.vector.tensor_add(prod[:, :, 0:16], prod[:, :, 0:16],
                                 prod[:, :, 16:32])
            nc.vector.tensor_add(prod[:, 96:256, 0:8], prod[:, 96:256, 0:8],
                                 prod[:, 96:256, 8:16])
            nc.gpsimd.tensor_add(prod[:, 0:96, 0:8], prod[:, 0:96, 0:8],
                                 prod[:, 0:96, 8:16])
            nc.gpsimd.tensor_add(prod[:, :, 0:4], prod[:, :, 0:4], prod[:, :, 4:8])
            nc.gpsimd.tensor_add(prod[:, :, 0:2], prod[:, :, 0:2], prod[:, :, 2:4])
            nc.gpsimd.tensor_add(ostage[:, 128:384], prod[:, :, 0], prod[:, :, 1])

            eng_out = nc.scalar if t % 2 == 0 else nc.sync
            eng_out.dma_start(out_d[n0:n0 + 128, :], ostage[:])

        # reps>1 is used only by the timing harness: the same (static) body is
        # unrolled `reps` times in one NEFF so per-dispatch overhead can be
        # subtracted out via a two-point slope.
        for _ in range(reps):
            pending = None
            for t in range(nt):
                cur = stage_a(t)
                if pending is not None:
                    stage_b(t - 1, *pending)
                pending = cur
            stage_b(nt - 1, *pending)

    return nc


def _prep_shared(film1_w, film1_b, fc1_w, film2_w, film2_b, inv_w, wemb_w):
    bf = ml_dtypes.bfloat16
    norm = np.float32(1.0 / np.sqrt(MUL))
    f1w = np.asarray(film1_w, np.float32); f1b = np.asarray(film1_b, np.float32)
    f2w = np.asarray(film2_w, np.float32); f2b = np.asarray(film2_b, np.float32)
    fcw = np.asarray(fc1_w, np.float32)
    invw = np.asarray(inv_w, np.float32)
    # wemb: fold norm; permute each irrep block from (i-major o) to (o-major i)
    wm = np.asarray(wemb_w, np.float32) * norm
    wp = np.empty_like(wm)
    wp[:, 0:1024] = wm[:, 0:1024].reshape(-1, MUL, MUL).transpose(0, 2, 1).reshape(-1, 1024)
    wp[:, 1024:2048] = wm[:, 1024:2048].reshape(-1, MUL, MUL).transpose(0, 2, 1).reshape(-1, 1024)
    # gammas with bias row: [65, 512] = (F1g; b1g) || (F2g; b2g)
    f12w = np.zeros((COND + 1, 512), np.float32)
    f12w[:COND, 0:256] = f1w[:, 0:S_IN]
    f12w[COND, 0:256] = f1b[0:S_IN]
    f12w[:COND, 256:512] = f2w[:, 0:S_IN]
    f12w[COND, 256:512] = f2b[0:S_IN]
    # beta folds (with bias rows), f32 host matmuls then bf16
    b1 = np.zeros((COND + 1, S_IN), np.float32)
    b1[:COND] = f1w[:, S_IN:]; b1[COND] = f1b[S_IN:]
    b2 = np.zeros((COND + 1, S_IN), np.float32)
    b2[:COND] = f2w[:, S_IN:]; b2[COND] = f2b[S_IN:]
    return {
        "f12w": f12w.astype(bf),
        "b1fc": (b1 @ fcw).astype(bf),
        "b2iv": (b2 @ invw).astype(bf),
        "b2wb": (b2 @ wp).astype(bf),
        "fcw": fcw.astype(bf),
        "invw": invw.astype(bf),
        "wembw": wp.astype(bf),
    }


def _prep_per_core(features, conditioning_tensor, nt=NT):
    """Pack activations into per-tile-contiguous blocks (see module docstring).

    Returns (xin, cTp): xin [NCORES, bpc, 512] bf16, cTp [NCORES, nt, 65, 128].
    """
    bf = ml_dtypes.bfloat16
    feats = np.asarray(features, np.float32)
    conds = np.asarray(conditioning_tensor, np.float32)
    ncr, bpc = NCORES, nt * 128

    # s0T block: [core, tile, p, (c b)] with value scalars[sample=n0+b, f=128c+p]
    sc = feats[:, :S_IN].astype(bf).reshape(ncr, nt, 128, 2, 128)  # [.., b, c, p]
    s0 = np.ascontiguousarray(sc.transpose(0, 1, 4, 3, 2)).reshape(ncr, nt, 128, 256)

    # xr block: equivariant features, (d-major, i) per irrep, per sample
    xe = np.empty((B, 256), bf)
    xe[:, 0:96] = feats[:, 256:352].reshape(-1, MUL, D1).transpose(0, 2, 1).reshape(-1, 96).astype(bf)
    xe[:, 96:256] = feats[:, 352:512].reshape(-1, MUL, D2).transpose(0, 2, 1).reshape(-1, 160).astype(bf)
    xr = xe.reshape(ncr, nt, 128, 256)

    xin = np.concatenate([s0, xr], axis=-1).reshape(ncr, bpc, 512)

    # cTp: [core, tile, 65, 128] with ones bias row
    cT = np.empty((COND + 1, B), bf)
    cT[:COND] = conds.T.astype(bf)
    cT[COND] = np.ones((B,), bf)
    cTp = np.ascontiguousarray(
        cT.reshape(COND + 1, ncr, nt, 128).transpose(1, 2, 0, 3))
    return np.ascontiguousarray(xin), cTp


def kernel(features, conditioning_tensor, film1_w, film1_b, fc1_w,
           film2_w, film2_b, inv_w, wemb_w):
    from concourse.bass_utils import run_bass_kernel_spmd

    if "nc" not in _cache:
        nc = _build()
        if not nc.is_finalized():
            nc.finalize()
        _cache["nc"] = nc
    nc = _cache["nc"]

    shared = _prep_shared(film1_w, film1_b, fc1_w, film2_w, film2_b, inv_w, wemb_w)
    xin, cTp = _prep_per_core(features, conditioning_tensor)

    in_maps = []
    for i in range(NCORES):
        m = dict(shared)
        m["xin"] = xin[i]
        m["cTp"] = cTp[i]
        in_maps.append(m)

    import os
    trace = bool(int(os.environ.get("KERNEL_TRACE", "0")))
    res = run_bass_kernel_spmd(nc, in_maps, core_ids=list(range(NCORES)), trace=trace)
    _cache["last"] = res
    return np.concatenate([r["out"] for r in res.results], axis=0).astype(np.float32)


# revision 13
# speedup vs baseline: 1.4163x; 1.0648x over previous
"""Trainium2 Bass kernel for nn_EquivariantScalarMLP (data-parallel over 8 cores).

Per-core layout (8192 samples, 64 tiles x 128 samples), feature-major chain:
  - Host pre-transposes the scalar block and pre-rearranges the equivariant
    block to (d-major, i-contiguous) bf16, then PACKS both into one
    per-tile-contiguous DRAM block (xin: [bpc, 512] where rows n0:n0+128 are
    tile t's [128 partition-lines x 1024 B]). The conditioning (+ones bias
    row) is packed per tile as cTp: [nt, 65, 128]. This keeps every
    per-tile DMA contiguous with >=512 B per partition line - the original
    column-gather layout (256 B / 130 B descriptors) made the kernel
    DMA-bound on hardware (~15 us/tile of DMA vs ~2 us modeled).
  - FiLM1/FiLM2 gammas/betas computed feature-major with film weights as the
    PE stationary; scalar chain s0T -> s1T -> s2T -> s3T stays feature-major,
    so fc1/inv/wemb all use s*T chunks directly as stationary operands.
  - Hypernet weights w = s3 @ wemb land batch-on-partition in PSUM; wemb
    columns are host-permuted to (o-major, i) per irrep block so the DVE mix
    runs at 2x: products [p,(o,d),i] via one broadcast mul per irrep, then an
    in-place log2 halving tree over i (bf16, innermost step-1 throughout).
  - e3nn 1/sqrt(32) norm folded into wemb on host; film biases folded via an
    extra ones row on condT (k=65 matmul).
"""

import numpy as np
import ml_dtypes

B = 65536
NCORES = 8
BPC = B // NCORES          # 8192 samples per core
NT = 64
S_IN = 256
S_OUT = 128
MUL = 32
COND = 64
D1, D2 = 3, 5

_cache = {}


def _build(nt=64, reps=1):
    from contextlib import ExitStack
    import concourse.bass as bass
    from concourse import bacc
    import concourse.tile as tile
    import concourse.mybir as mybir

    bpc = nt * 128
    f32 = mybir.dt.float32
    bf16 = mybir.dt.bfloat16
    AF = mybir.ActivationFunctionType

    nc = bacc.Bacc()

    xin_d = nc.dram_tensor("xin", [bpc, 512], bf16, kind="ExternalInput")
    cTp_d = nc.dram_tensor("cTp", [nt, COND + 1, 128], bf16, kind="ExternalInput")
    f12w_d = nc.dram_tensor("f12w", [COND + 1, 512], bf16, kind="ExternalInput")
    b1fc_d = nc.dram_tensor("b1fc", [COND + 1, 256], bf16, kind="ExternalInput")
    b2iv_d = nc.dram_tensor("b2iv", [COND + 1, 128], bf16, kind="ExternalInput")
    b2wb_d = nc.dram_tensor("b2wb", [COND + 1, 2048], bf16, kind="ExternalInput")
    fcw_d = nc.dram_tensor("fcw", [S_IN, S_IN], bf16, kind="ExternalInput")
    invw_d = nc.dram_tensor("invw", [S_IN, S_OUT], bf16, kind="ExternalInput")
    wembw_d = nc.dram_tensor("wembw", [S_IN, 2048], bf16, kind="ExternalInput")
    out_d = nc.dram_tensor("out", [bpc, 384], bf16, kind="ExternalOutput")

    with tile.TileContext(nc) as tc, ExitStack() as ctx:
        wpool = ctx.enter_context(tc.tile_pool(name="weights", bufs=1))
        # --- persistent weights in SBUF ---
        f12w = wpool.tile([COND + 1, 512], bf16)
        nc.sync.dma_start(f12w[:], f12w_d[:])
        b1fc = wpool.tile([COND + 1, 256], bf16)
        nc.sync.dma_start(b1fc[:], b1fc_d[:])
        b2iv = wpool.tile([COND + 1, 128], bf16)
        nc.sync.dma_start(b2iv[:], b2iv_d[:])
        b2wb = wpool.tile([COND + 1, 2048], bf16)
        nc.sync.dma_start(b2wb[:], b2wb_d[:])
        # k-major stores: chunk c holds rows [128c, 128c+128) of the weight
        fcw = wpool.tile([128, 512], bf16)
        nc.sync.dma_start(fcw[:].rearrange("k (c m) -> k c m", c=2),
                          fcw_d.rearrange("(c k) m -> k c m", c=2))
        invw = wpool.tile([128, 256], bf16)
        nc.sync.dma_start(invw[:].rearrange("k (c m) -> k c m", c=2),
                          invw_d.rearrange("(c k) m -> k c m", c=2))
        wemb = wpool.tile([128, 4096], bf16)
        nc.sync.dma_start(wemb[:].rearrange("k (c m) -> k c m", c=2),
                          wembw_d.rearrange("(c k) m -> k c m", c=2))

        inp = ctx.enter_context(tc.tile_pool(name="inp", bufs=6))
        act = ctx.enter_context(tc.tile_pool(name="act", bufs=2))
        wsbp = ctx.enter_context(tc.tile_pool(name="wsbp", bufs=3))
        prodp = ctx.enter_context(tc.tile_pool(name="prodp", bufs=3))
        outp = ctx.enter_context(tc.tile_pool(name="outp", bufs=6))
        # PSUM pools (bank-granular: 8 x 2KB per partition total)
        ps_gb = ctx.enter_context(tc.tile_pool(name="ps_gb", bufs=1, space="PSUM"))
        ps_s2 = ctx.enter_context(tc.tile_pool(name="ps_s2", bufs=1, space="PSUM"))
        ps_iv = ctx.enter_context(tc.tile_pool(name="ps_iv", bufs=1, space="PSUM"))
        ps_w = ctx.enter_context(tc.tile_pool(name="ps_w", bufs=1, space="PSUM"))

        def stage_a(t):
            n0 = t * 128
            # one contiguous 128 KiB load: cols 0:256 = s0T (feature-major
            # scalars, [p, c, b] with f = 128c+p), cols 256:512 = xr
            xin = inp.tile([128, 512], bf16, tag="xin")
            nc.sync.dma_start(xin[:], xin_d[n0:n0 + 128, :])
            s0T = xin[:, 0:256].rearrange("p (c b) -> p c b", c=2)
            xr = xin[:, 256:512]
            cT = inp.tile([COND + 1, 128], bf16, tag="cT")
            nc.sync.dma_start(cT[:], cTp_d[t])

            # --- films feature-major, gammas only (betas folded on host) ---
            gb = ps_gb.tile([128, 512], f32, tag="gb")
            for m in range(4):
                nc.tensor.matmul(gb[:, 128 * m:128 * (m + 1)],
                                 f12w[:, 128 * m:128 * (m + 1)], cT[:],
                                 start=True, stop=True)
            gam1 = act.tile([128, 256], bf16, tag="gam1")
            gam2 = act.tile([128, 256], bf16, tag="gam2")
            nc.scalar.activation(gam1[:], gb[:, 0:256], AF.Copy)
            nc.scalar.activation(gam2[:], gb[:, 256:512], AF.Copy)

            # --- FiLM1 gamma on DVE; beta1 enters via b1fc matmul below ---
            s1T = act.tile([128, 256], bf16, tag="s1T")
            nc.gpsimd.tensor_mul(s1T[:], s0T.rearrange("p c b -> p (c b)"), gam1[:])

            # --- fc1 + folded beta1: s2T[m=feat, b] ---
            s2ps = ps_s2.tile([128, 256], f32, tag="s2ps")
            for m in range(2):
                nc.tensor.matmul(s2ps[:, 128 * m:128 * (m + 1)],
                                 b1fc[:, 128 * m:128 * (m + 1)], cT[:],
                                 start=True, stop=False)
                for c in range(2):
                    nc.tensor.matmul(s2ps[:, 128 * m:128 * (m + 1)],
                                     fcw[:, 256 * c + 128 * m:256 * c + 128 * (m + 1)],
                                     s1T[:, 128 * c:128 * (c + 1)],
                                     start=False, stop=(c == 1))
            s2T = act.tile([128, 256], bf16, tag="s2T")
            nc.scalar.activation(s2T[:], s2ps[:], AF.Copy)

            # --- FiLM2 gamma on DVE; beta2 folded into inv/wemb below ---
            s3T = act.tile([128, 256], bf16, tag="s3T")
            nc.gpsimd.tensor_mul(s3T[:], s2T[:], gam2[:])

            ostage = outp.tile([128, 384], bf16, tag="ostage")

            # --- inv (batch-major) + hypernet w, sharing s3T stationary ---
            invps = ps_iv.tile([128, 128], f32, tag="invps")
            wps0 = ps_w.tile([128, 1024], f32, tag="wps0")
            wps1 = ps_w.tile([128, 1024], f32, tag="wps1")
            wps = [wps0, wps1]
            nc.tensor.matmul(invps[:], cT[:], b2iv[:], start=True, stop=False)
            for h in range(2):
                for q in range(2):
                    nc.tensor.matmul(wps[h][:, 512 * q:512 * (q + 1)], cT[:],
                                     b2wb[:, 1024 * h + 512 * q:1024 * h + 512 * (q + 1)],
                                     start=True, stop=False)
            for c in range(2):
                nc.tensor.matmul(invps[:], s3T[:, 128 * c:128 * (c + 1)],
                                 invw[:, 128 * c:128 * (c + 1)],
                                 start=False, stop=(c == 1))
                for h in range(2):
                    for q in range(2):
                        nc.tensor.matmul(
                            wps[h][:, 512 * q:512 * (q + 1)],
                            s3T[:, 128 * c:128 * (c + 1)],
                            wemb[:, 2048 * c + 1024 * h + 512 * q:
                                 2048 * c + 1024 * h + 512 * (q + 1)],
                            start=False, stop=(c == 1))
            nc.scalar.activation(ostage[:, 0:128], invps[:], AF.Copy)
            wsb = wsbp.tile([128, 2048], bf16, tag="wsb")
            nc.scalar.activation(wsb[:, 0:1024], wps0[:], AF.Copy)
            nc.scalar.activation(wsb[:, 1024:2048], wps1[:], AF.Copy)
            return wsb, xr, ostage

        def stage_b(t, wsb, xr, ostage):
            n0 = t * 128
            # --- equivariant mix on DVE: products then halving tree over i ---
            # prod rows: 96 = (o,d) for l=1, then 160 = (o,d) for l=2
            # engine split balances DVE (~0.54 ns/elem) vs Pool (~0.92):
            # DVE: products l2 + step1(all) + step2-l2; Pool: products l1 +
            # step2-l1 + step3 + step4 + final (plus s1T/s3T in stage_a)
            prod = prodp.tile([128, 256, 32], bf16, tag="prod")
            w1v = wsb[:, 0:1024].rearrange("p (o i) -> p o i", o=32).unsqueeze(2).broadcast_to((128, 32, 3, 32))
            x1v = xr[:, 0:96].rearrange("p (d i) -> p d i", d=3).unsqueeze(1).broadcast_to((128, 32, 3, 32))
            nc.vector.tensor_mul(prod[:, 0:96, :].rearrange("p (o d) i -> p o d i", o=32), w1v, x1v)
            w2v = wsb[:, 1024:2048].rearrange("p (o i) -> p o i", o=32).unsqueeze(2).broadcast_to((128, 32, 5, 32))
            x2v = xr[:, 96:256].rearrange("p (d i) -> p d i", d=5).unsqueeze(1).broadcast_to((128, 32, 5, 32))
            nc.vector.tensor_mul(prod[:, 96:256, :].rearrange("p (o d) i -> p o d i", o=32), w2v, x2v)
            nc.vector.tensor_add(prod[:, :, 0:16], prod[:, :, 0:16],
                                 prod[:, :, 16:32])
            nc.vector.tensor_add(prod[:, 96:256, 0:8], prod[:, 96:256, 0:8],
                                 prod[:, 96:256, 8:16])
            nc.gpsimd.tensor_add(prod[:, 0:96, 0:8], prod[:, 0:96, 0:8],
                                 prod[:, 0:96, 8:16])
            nc.gpsimd.tensor_add(prod[:, :, 0:4], prod[:, :, 0:4], prod[:, :, 4:8])
            nc.gpsimd.tensor_add(prod[:, :, 0:2], prod[:, :, 0:2], prod[:, :, 2:4])
            nc.gpsimd.tensor_add(ostage[:, 128:384], prod[:, :, 0], prod[:, :, 1])

            nc.scalar.dma_start(out_d[n0:n0 + 128, :], ostage[:])

        # reps>1 is used only by the timing harness: the same (static) body is
        # unrolled `reps` times in one NEFF so per-dispatch overhead can be
        # subtracted out via a two-point slope.
        for _ in range(reps):
            pending = None
            for t in range(nt):
                cur = stage_a(t)
                if pending is not None:
                    stage_b(t - 1, *pending)
                pending = cur
            stage_b(nt - 1, *pending)

    return nc


def _prep_shared(film1_w, film1_b, fc1_w, film2_w, film2_b, inv_w, wemb_w):
    bf = ml_dtypes.bfloat16
    norm = np.float32(1.0 / np.sqrt(MUL))
    f1w = np.asarray(film1_w, np.float32); f1b = np.asarray(film1_b, np.float32)
    f2w = np.asarray(film2_w, np.float32); f2b = np.asarray(film2_b, np.float32)
    fcw = np.asarray(fc1_w, np.float32)
    invw = np.asarray(inv_w, np.float32)
    # wemb: fold norm; permute each irrep block from (i-major o) to (o-major i)
    wm = np.asarray(wemb_w, np.float32) * norm
    wp = np.empty_like(wm)
    wp[:, 0:1024] = wm[:, 0:1024].reshape(-1, MUL, MUL).transpose(0, 2, 1).reshape(-1, 1024)
    wp[:, 1024:2048] = wm[:, 1024:2048].reshape(-1, MUL, MUL).transpose(0, 2, 1).reshape(-1, 1024)
    # gammas with bias row: [65, 512] = (F1g; b1g) || (F2g; b2g)
    f12w = np.zeros((COND + 1, 512), np.float32)
    f12w[:COND, 0:256] = f1w[:, 0:S_IN]
    f12w[COND, 0:256] = f1b[0:S_IN]
    f12w[:COND, 256:512] = f2w[:, 0:S_IN]
    f12w[COND, 256:512] = f2b[0:S_IN]
    # beta folds (with bias rows), f32 host matmuls then bf16
    b1 = np.zeros((COND + 1, S_IN), np.float32)
    b1[:COND] = f1w[:, S_IN:]; b1[COND] = f1b[S_IN:]
    b2 = np.zeros((COND + 1, S_IN), np.float32)
    b2[:COND] = f2w[:, S_IN:]; b2[COND] = f2b[S_IN:]
    return {
        "f12w": f12w.astype(bf),
        "b1fc": (b1 @ fcw).astype(bf),
        "b2iv": (b2 @ invw).astype(bf),
        "b2wb": (b2 @ wp).astype(bf),
        "fcw": fcw.astype(bf),
        "invw": invw.astype(bf),
        "wembw": wp.astype(bf),
    }


def _prep_per_core(features, conditioning_tensor, nt=NT):
    """Pack activations into per-tile-contiguous blocks (see module docstring).

    Returns (xin, cTp): xin [NCORES, bpc, 512] bf16, cTp [NCORES, nt, 65, 128].
    """
    bf = ml_dtypes.bfloat16
    feats = np.asarray(features, np.float32)
    conds = np.asarray(conditioning_tensor, np.float32)
    ncr, bpc = NCORES, nt * 128

    # s0T block: [core, tile, p, (c b)] with value scalars[sample=n0+b, f=128c+p]
    sc = feats[:, :S_IN].astype(bf).reshape(ncr, nt, 128, 2, 128)  # [.., b, c, p]
    s0 = np.ascontiguousarray(sc.transpose(0, 1, 4, 3, 2)).reshape(ncr, nt, 128, 256)

    # xr block: equivariant features, (d-major, i) per irrep, per sample
    xe = np.empty((B, 256), bf)
    xe[:, 0:96] = feats[:, 256:352].reshape(-1, MUL, D1).transpose(0, 2, 1).reshape(-1, 96).astype(bf)
    xe[:, 96:256] = feats[:, 352:512].reshape(-1, MUL, D2).transpose(0, 2, 1).reshape(-1, 160).astype(bf)
    xr = xe.reshape(ncr, nt, 128, 256)

    xin = np.concatenate([s0, xr], axis=-1).reshape(ncr, bpc, 512)

    # cTp: [core, tile, 65, 128] with ones bias row
    cT = np.empty((COND + 1, B), bf)
    cT[:COND] = conds.T.astype(bf)
    cT[COND] = np.ones((B,), bf)
    cTp = np.ascontiguousarray(
        cT.reshape(COND + 1, ncr, nt, 128).transpose(1, 2, 0, 3))
    return np.ascontiguousarray(xin), cTp


def kernel(features, conditioning_tensor, film1_w, film1_b, fc1_w,
           film2_w, film2_b, inv_w, wemb_w):
    from concourse.bass_utils import run_bass_kernel_spmd

    if "nc" not in _cache:
        nc = _build()
        if not nc.is_finalized():
            nc.finalize()
        _cache["nc"] = nc
    nc = _cache["nc"]

    shared = _prep_shared(film1_w, film1_b, fc1_w, film2_w, film2_b, inv_w, wemb_w)
    xin, cTp = _prep_per_core(features, conditioning_tensor)

    in_maps = []
    for i in range(NCORES):
        m = dict(shared)
        m["xin"] = xin[i]
        m["cTp"] = cTp[i]
        in_maps.append(m)

    import os
    trace = bool(int(os.environ.get("KERNEL_TRACE", "0")))
    res = run_bass_kernel_spmd(nc, in_maps, core_ids=list(range(NCORES)), trace=trace)
    _cache["last"] = res
    return np.concatenate([r["out"] for r in res.results], axis=0).astype(np.float32)


# revision 19
# speedup vs baseline: 1.4605x; 1.0312x over previous
"""Trainium2 Bass kernel for nn_EquivariantScalarMLP (data-parallel over 8 cores).

Per-core layout (8192 samples, 64 tiles x 128 samples), feature-major chain:
  - Host pre-transposes the scalar block and pre-rearranges the equivariant
    block to (d-major, i-contiguous) bf16, then PACKS both into one
    per-tile-contiguous DRAM block (xin: [bpc, 512] where rows n0:n0+128 are
    tile t's [128 partition-lines x 1024 B]). The conditioning (+ones bias
    row) is packed per tile as cTp: [nt, 65, 128]. This keeps every
    per-tile DMA contiguous with >=512 B per partition line - the original
    column-gather layout (256 B / 130 B descriptors) made the kernel
    DMA-bound on hardware (~15 us/tile of DMA vs ~2 us modeled).
  - FiLM1/FiLM2 gammas/betas computed feature-major with film weights as the
    PE stationary; scalar chain s0T -> s1T -> s2T -> s3T stays feature-major,
    so fc1/inv/wemb all use s*T chunks directly as stationary operands.
  - Hypernet weights w = s3 @ wemb land batch-on-partition in PSUM; wemb
    columns are host-permuted to (o-major, i) per irrep block so the DVE mix
    runs at 2x: products [p,(o,d),i] via one broadcast mul per irrep, then an
    in-place log2 halving tree over i (bf16, innermost step-1 throughout).
  - e3nn 1/sqrt(32) norm folded into wemb on host; film biases folded via an
    extra ones row on condT (k=65 matmul).
"""

import numpy as np
import ml_dtypes

B = 65536
NCORES = 8
BPC = B // NCORES          # 8192 samples per core
NT = 64
S_IN = 256
S_OUT = 128
MUL = 32
COND = 64
D1, D2 = 3, 5
GDMA = 4

_cache = {}


def _build(nt=64, reps=1, gdma=GDMA):
    from contextlib import ExitStack
    import concourse.bass as bass
    from concourse import bacc
    import concourse.tile as tile
    import concourse.mybir as mybir

    bpc = nt * 128
    f32 = mybir.dt.float32
    bf16 = mybir.dt.bfloat16
    AF = mybir.ActivationFunctionType

    nc = bacc.Bacc()

    ngroups = nt // gdma
    xin_d = nc.dram_tensor("xin", [ngroups * 128, gdma * 512], bf16, kind="ExternalInput")
    cTp_d = nc.dram_tensor("cTp", [ngroups, COND + 1, gdma * 128], bf16, kind="ExternalInput")
    f12w_d = nc.dram_tensor("f12w", [COND + 1, 512], bf16, kind="ExternalInput")
    b1fc_d = nc.dram_tensor("b1fc", [COND + 1, 256], bf16, kind="ExternalInput")
    b2iv_d = nc.dram_tensor("b2iv", [COND + 1, 128], bf16, kind="ExternalInput")
    b2wb_d = nc.dram_tensor("b2wb", [COND + 1, 2048], bf16, kind="ExternalInput")
    fcw_d = nc.dram_tensor("fcw", [S_IN, S_IN], bf16, kind="ExternalInput")
    invw_d = nc.dram_tensor("invw", [S_IN, S_OUT], bf16, kind="ExternalInput")
    wembw_d = nc.dram_tensor("wembw", [S_IN, 2048], bf16, kind="ExternalInput")
    out_d = nc.dram_tensor("out", [ngroups * 128, gdma * 384], bf16, kind="ExternalOutput")

    with tile.TileContext(nc) as tc, ExitStack() as ctx:
        wpool = ctx.enter_context(tc.tile_pool(name="weights", bufs=1))
        # --- persistent weights in SBUF ---
        f12w = wpool.tile([COND + 1, 512], bf16)
        nc.sync.dma_start(f12w[:], f12w_d[:])
        b1fc = wpool.tile([COND + 1, 256], bf16)
        nc.sync.dma_start(b1fc[:], b1fc_d[:])
        b2iv = wpool.tile([COND + 1, 128], bf16)
        nc.sync.dma_start(b2iv[:], b2iv_d[:])
        b2wb = wpool.tile([COND + 1, 2048], bf16)
        nc.sync.dma_start(b2wb[:], b2wb_d[:])
        # k-major stores: chunk c holds rows [128c, 128c+128) of the weight
        fcw = wpool.tile([128, 512], bf16)
        nc.sync.dma_start(fcw[:].rearrange("k (c m) -> k c m", c=2),
                          fcw_d.rearrange("(c k) m -> k c m", c=2))
        invw = wpool.tile([128, 256], bf16)
        nc.sync.dma_start(invw[:].rearrange("k (c m) -> k c m", c=2),
                          invw_d.rearrange("(c k) m -> k c m", c=2))
        wemb = wpool.tile([128, 4096], bf16)
        nc.sync.dma_start(wemb[:].rearrange("k (c m) -> k c m", c=2),
                          wembw_d.rearrange("(c k) m -> k c m", c=2))

        inp = ctx.enter_context(tc.tile_pool(name="inp", bufs=6))
        act = ctx.enter_context(tc.tile_pool(name="act", bufs=2))
        wsbp = ctx.enter_context(tc.tile_pool(name="wsbp", bufs=3))
        prodp = ctx.enter_context(tc.tile_pool(name="prodp", bufs=3))
        outp = ctx.enter_context(tc.tile_pool(name="outp", bufs=6))
        # PSUM pools (bank-granular: 8 x 2KB per partition total)
        ps_gb = ctx.enter_context(tc.tile_pool(name="ps_gb", bufs=1, space="PSUM"))
        ps_s2 = ctx.enter_context(tc.tile_pool(name="ps_s2", bufs=1, space="PSUM"))
        ps_iv = ctx.enter_context(tc.tile_pool(name="ps_iv", bufs=1, space="PSUM"))
        ps_w = ctx.enter_context(tc.tile_pool(name="ps_w", bufs=1, space="PSUM"))

        def stage_a(t, xin, cT, ostage):
            # xin cols 0:256 = s0T (feature-major scalars, [p, c, b] with
            # f = 128c+p), cols 256:512 = xr
            s0T = xin[:, 0:256].rearrange("p (c b) -> p c b", c=2)
            xr = xin[:, 256:512]

            # --- films feature-major, gammas only (betas folded on host) ---
            gb = ps_gb.tile([128, 512], f32, tag="gb")
            for m in range(4):
                nc.tensor.matmul(gb[:, 128 * m:128 * (m + 1)],
                                 f12w[:, 128 * m:128 * (m + 1)], cT[:],
                                 start=True, stop=True)
            gam1 = act.tile([128, 256], bf16, tag="gam1")
            gam2 = act.tile([128, 256], bf16, tag="gam2")
            nc.scalar.activation(gam1[:], gb[:, 0:256], AF.Copy)
            nc.scalar.activation(gam2[:], gb[:, 256:512], AF.Copy)

            # --- FiLM1 gamma on DVE; beta1 enters via b1fc matmul below ---
            s1T = act.tile([128, 256], bf16, tag="s1T")
            nc.gpsimd.tensor_mul(s1T[:], s0T.rearrange("p c b -> p (c b)"), gam1[:])

            # --- fc1 + folded beta1: s2T[m=feat, b] ---
            s2ps = ps_s2.tile([128, 256], f32, tag="s2ps")
            for m in range(2):
                nc.tensor.matmul(s2ps[:, 128 * m:128 * (m + 1)],
                                 b1fc[:, 128 * m:128 * (m + 1)], cT[:],
                                 start=True, stop=False)
                for c in range(2):
                    nc.tensor.matmul(s2ps[:, 128 * m:128 * (m + 1)],
                                     fcw[:, 256 * c + 128 * m:256 * c + 128 * (m + 1)],
                                     s1T[:, 128 * c:128 * (c + 1)],
                                     start=False, stop=(c == 1))
            s2T = act.tile([128, 256], bf16, tag="s2T")
            nc.scalar.activation(s2T[:], s2ps[:], AF.Copy)

            # --- FiLM2 gamma on DVE; beta2 folded into inv/wemb below ---
            s3T = act.tile([128, 256], bf16, tag="s3T")
            nc.gpsimd.tensor_mul(s3T[:], s2T[:], gam2[:])

            # --- inv (batch-major) + hypernet w, sharing s3T stationary ---
            invps = ps_iv.tile([128, 128], f32, tag="invps")
            wps0 = ps_w.tile([128, 1024], f32, tag="wps0")
            wps1 = ps_w.tile([128, 1024], f32, tag="wps1")
            wps = [wps0, wps1]
            nc.tensor.matmul(invps[:], cT[:], b2iv[:], start=True, stop=False)
            for h in range(2):
                for q in range(2):
                    nc.tensor.matmul(wps[h][:, 512 * q:512 * (q + 1)], cT[:],
                                     b2wb[:, 1024 * h + 512 * q:1024 * h + 512 * (q + 1)],
                                     start=True, stop=False)
            for c in range(2):
                nc.tensor.matmul(invps[:], s3T[:, 128 * c:128 * (c + 1)],
                                 invw[:, 128 * c:128 * (c + 1)],
                                 start=False, stop=(c == 1))
                for h in range(2):
                    for q in range(2):
                        nc.tensor.matmul(
                            wps[h][:, 512 * q:512 * (q + 1)],
                            s3T[:, 128 * c:128 * (c + 1)],
                            wemb[:, 2048 * c + 1024 * h + 512 * q:
                                 2048 * c + 1024 * h + 512 * (q + 1)],
                            start=False, stop=(c == 1))
            nc.scalar.activation(ostage[:, 0:128], invps[:], AF.Copy)
            wsb = wsbp.tile([128, 2048], bf16, tag="wsb")
            nc.scalar.activation(wsb[:, 0:1024], wps0[:], AF.Copy)
            nc.scalar.activation(wsb[:, 1024:2048], wps1[:], AF.Copy)
            return wsb, xr, ostage

        def stage_b(t, wsb, xr, ostage):
            # --- equivariant mix on DVE: products then halving tree over i ---
            # prod rows: 96 = (o,d) for l=1, then 160 = (o,d) for l=2
            # engine split balances DVE (~0.54 ns/elem) vs Pool (~0.92):
            # DVE: products l2 + step1(all) + step2-l2; Pool: products l1 +
            # step2-l1 + step3 + step4 + final (plus s1T/s3T in stage_a)
            prod = prodp.tile([128, 256, 32], bf16, tag="prod")
            w1v = wsb[:, 0:1024].rearrange("p (o i) -> p o i", o=32).unsqueeze(2).broadcast_to((128, 32, 3, 32))
            x1v = xr[:, 0:96].rearrange("p (d i) -> p d i", d=3).unsqueeze(1).broadcast_to((128, 32, 3, 32))
            nc.vector.tensor_mul(prod[:, 0:96, :].rearrange("p (o d) i -> p o d i", o=32), w1v, x1v)
            w2v = wsb[:, 1024:2048].rearrange("p (o i) -> p o i", o=32).unsqueeze(2).broadcast_to((128, 32, 5, 32))
            x2v = xr[:, 96:256].rearrange("p (d i) -> p d i", d=5).unsqueeze(1).broadcast_to((128, 32, 5, 32))
            nc.vector.tensor_mul(prod[:, 96:256, :].rearrange("p (o d) i -> p o d i", o=32), w2v, x2v)
            nc.vector.tensor_add(prod[:, :, 0:16], prod[:, :, 0:16],
                                 prod[:, :, 16:32])
            nc.vector.tensor_add(prod[:, 96:256, 0:8], prod[:, 96:256, 0:8],
                                 prod[:, 96:256, 8:16])
            nc.gpsimd.tensor_add(prod[:, 0:96, 0:8], prod[:, 0:96, 0:8],
                                 prod[:, 0:96, 8:16])
            nc.gpsimd.tensor_add(prod[:, :, 0:4], prod[:, :, 0:4], prod[:, :, 4:8])
            nc.gpsimd.tensor_add(prod[:, :, 0:2], prod[:, :, 0:2], prod[:, :, 2:4])
            nc.gpsimd.tensor_add(ostage[:, 128:384], prod[:, :, 0], prod[:, :, 1])

        # DMA is grouped G tiles per dma_start (contiguous blocks): per-tile
        # dma_starts were the HW bottleneck regardless of descriptor shape,
        # so cut the instruction count 4x while keeping descriptors/instr
        # moderate (the all-64-tiles-in-one variant crashed the NEFF).
        G = gdma
        xin_g = xin_d.rearrange("(g p) (j c) -> g p j c", p=128, j=G)
        out_g = out_d.rearrange("(g p) (j c) -> g p j c", p=128, j=G)

        def load_group(g):
            xin2 = inp.tile([128, G, 512], bf16, tag="xin")
            nc.sync.dma_start(xin2[:], xin_g[g])
            cT2 = inp.tile([COND + 1, G, 128], bf16, tag="cT")
            nc.sync.dma_start(cT2[:], cTp_d[g].rearrange("p (j b) -> p j b", j=G))
            ostage2 = outp.tile([128, G, 384], bf16, tag="ostage")
            return xin2, cT2, ostage2

        def store_group(g, ostage2):
            nc.scalar.dma_start(out_g[g], ostage2[:])

        # reps>1 is used only by the timing harness: the same (static) body is
        # unrolled `reps` times in one NEFF so per-dispatch overhead can be
        # subtracted out via a two-point slope.
        for _ in range(reps):
            pending = None
            grp = {}
            for t in range(nt):
                g, j = divmod(t, G)
                if j == 0:
                    grp[g] = load_group(g)
                xin2, cT2, ostage2 = grp[g]
                cur = (t, stage_a(t, xin2[:, j, :], cT2[:, j, :],
                                  ostage2[:, j, :]), ostage2)
                if pending is not None:
                    tp, ab, osp = pending
                    stage_b(tp, *ab)
                    if (tp + 1) % G == 0:
                        store_group(tp // G, osp)
                        del grp[tp // G]
                pending = cur
            tp, ab, osp = pending
            stage_b(tp, *ab)
            store_group(tp // G, osp)

    return nc


def _prep_shared(film1_w, film1_b, fc1_w, film2_w, film2_b, inv_w, wemb_w):
    bf = ml_dtypes.bfloat16
    norm = np.float32(1.0 / np.sqrt(MUL))
    f1w = np.asarray(film1_w, np.float32); f1b = np.asarray(film1_b, np.float32)
    f2w = np.asarray(film2_w, np.float32); f2b = np.asarray(film2_b, np.float32)
    fcw = np.asarray(fc1_w, np.float32)
    invw = np.asarray(inv_w, np.float32)
    # wemb: fold norm; permute each irrep block from (i-major o) to (o-major i)
    wm = np.asarray(wemb_w, np.float32) * norm
    wp = np.empty_like(wm)
    wp[:, 0:1024] = wm[:, 0:1024].reshape(-1, MUL, MUL).transpose(0, 2, 1).reshape(-1, 1024)
    wp[:, 1024:2048] = wm[:, 1024:2048].reshape(-1, MUL, MUL).transpose(0, 2, 1).reshape(-1, 1024)
    # gammas with bias row: [65, 512] = (F1g; b1g) || (F2g; b2g)
    f12w = np.zeros((COND + 1, 512), np.float32)
    f12w[:COND, 0:256] = f1w[:, 0:S_IN]
    f12w[COND, 0:256] = f1b[0:S_IN]
    f12w[:COND, 256:512] = f2w[:, 0:S_IN]
    f12w[COND, 256:512] = f2b[0:S_IN]
    # beta folds (with bias rows), f32 host matmuls then bf16
    b1 = np.zeros((COND + 1, S_IN), np.float32)
    b1[:COND] = f1w[:, S_IN:]; b1[COND] = f1b[S_IN:]
    b2 = np.zeros((COND + 1, S_IN), np.float32)
    b2[:COND] = f2w[:, S_IN:]; b2[COND] = f2b[S_IN:]
    return {
        "f12w": f12w.astype(bf),
        "b1fc": (b1 @ fcw).astype(bf),
        "b2iv": (b2 @ invw).astype(bf),
        "b2wb": (b2 @ wp).astype(bf),
        "fcw": fcw.astype(bf),
        "invw": invw.astype(bf),
        "wembw": wp.astype(bf),
    }


def _prep_per_core(features, conditioning_tensor, nt=NT):
    """Pack activations into per-tile-contiguous blocks (see module docstring).

    Returns (xin, cTp): xin [NCORES, bpc, 512] bf16, cTp [NCORES, nt, 65, 128].
    """
    bf = ml_dtypes.bfloat16
    feats = np.asarray(features, np.float32)
    conds = np.asarray(conditioning_tensor, np.float32)
    ncr, bpc = NCORES, nt * 128

    # s0T block: [core, tile, p, (c b)] with value scalars[sample=n0+b, f=128c+p]
    sc = feats[:, :S_IN].astype(bf).reshape(ncr, nt, 128, 2, 128)  # [.., b, c, p]
    s0 = np.ascontiguousarray(sc.transpose(0, 1, 4, 3, 2)).reshape(ncr, nt, 128, 256)

    # xr block: equivariant features, (d-major, i) per irrep, per sample
    xe = np.empty((B, 256), bf)
    xe[:, 0:96] = feats[:, 256:352].reshape(-1, MUL, D1).transpose(0, 2, 1).reshape(-1, 96).astype(bf)
    xe[:, 96:256] = feats[:, 352:512].reshape(-1, MUL, D2).transpose(0, 2, 1).reshape(-1, 160).astype(bf)
    xr = xe.reshape(ncr, nt, 128, 256)

    xin = np.concatenate([s0, xr], axis=-1)          # [ncr, nt, 128, 512]
    ngroups = nt // GDMA
    xin = np.ascontiguousarray(
        xin.reshape(ncr, ngroups, GDMA, 128, 512).transpose(0, 1, 3, 2, 4)
    ).reshape(ncr, ngroups * 128, GDMA * 512)

    # cTp: [core, tile, 65, 128] with ones bias row
    cT = np.empty((COND + 1, B), bf)
    cT[:COND] = conds.T.astype(bf)
    cT[COND] = np.ones((B,), bf)
    cTp = cT.reshape(COND + 1, ncr, nt, 128).transpose(1, 2, 0, 3)
    cTp = np.ascontiguousarray(
        cTp.reshape(ncr, ngroups, GDMA, COND + 1, 128).transpose(0, 1, 3, 2, 4)
    ).reshape(ncr, ngroups, COND + 1, GDMA * 128)
    return xin, cTp


def kernel(features, conditioning_tensor, film1_w, film1_b, fc1_w,
           film2_w, film2_b, inv_w, wemb_w):
    from concourse.bass_utils import run_bass_kernel_spmd

    if "nc" not in _cache:
        nc = _build()
        if not nc.is_finalized():
            nc.finalize()
        _cache["nc"] = nc
    nc = _cache["nc"]

    shared = _prep_shared(film1_w, film1_b, fc1_w, film2_w, film2_b, inv_w, wemb_w)
    xin, cTp = _prep_per_core(features, conditioning_tensor)

    in_maps = []
    for i in range(NCORES):
        m = dict(shared)
        m["xin"] = xin[i]
        m["cTp"] = cTp[i]
        in_maps.append(m)

    import os
    trace = bool(int(os.environ.get("KERNEL_TRACE", "0")))
    res = run_bass_kernel_spmd(nc, in_maps, core_ids=list(range(NCORES)), trace=trace)
    _cache["last"] = res
    ngroups = NT // GDMA
    outs = []
    for r in res.results:
        o = r["out"].reshape(ngroups, 128, GDMA, 384).transpose(0, 2, 1, 3)
        outs.append(o.reshape(BPC, 384))
    return np.concatenate(outs, axis=0).astype(np.float32)
